# revision 13
# baseline (speedup 1.0000x reference)
"""Llama GQA attention block (B=1, S=2048, D=4096, 32 Q heads / 8 KV heads,
hd=128) on 8 trn2 NeuronCores — v2.

Sharding: tensor-parallel by attention head. Core c owns q-heads 4c..4c+3 and
kv-head c (one GQA group). It computes the QKV projection for its 768 rows of
wqkv, RoPE, attention, then a K=512 PARTIAL output projection using only its
own attention output (y columns 512c..512c+512 against the matching 512 rows
of wo^T). Partials are summed across cores with a per-sq-block ReduceScatter
that overlaps the next block's compute; the host reassembles the row shards.

Key design points (vs the v1 baseline that AllGathered y and
column-sharded wo; sim cost model: 851us -> 413us):
  - No AllGather barrier, no 16MB y DRAM round-trip: y stays in SBUF and
    each per-block ReduceScatter overlaps the next block's compute on the
    collective cores. Only the last block's RS (~28us) is a serial tail.
  - Generator-driven instruction interleave: each phase round-robins the
    emission of attn(sb) with qkv(sb+1) and outproj(sb-1) so the in-order
    PE queue always has independent matmuls to fill exp/normalize latency
    gaps (also keeps the PE p-state ramped at full clock).
  - The Pool/gpsimd queue carries ONLY collectives + their bounce DMAs:
    a 28us ReduceScatter head-of-line blocks everything behind it on that
    engine, so no latency-critical work may queue there.
  - QKV PSUM rotates 2 banks (e-major loop over a pre-staged 4MB x block);
    PSUM budget = qkv 2 + scores 2 + yacc 2 + outproj/denominator 2 banks.
  - RoPE rotate-half via SBUF->SBUF DMA (sign folded into the sin table);
    rope math in bf16 on DVE (2x mode); 1/sqrt(hd) folded into wq; cos/sin
    tables shared between q and k.
  - mask applied multiplicatively AFTER exp (exp(s+m) = exp(s)*exp(m),
    exp(mask) precomputed on host in bf16): fully-masked [128,512] tiles
    skipped, all-zero tiles free, partially-masked causal-diagonal tiles
    narrowed to their live column range (compute, exp and dacc adds all
    shrink). Handles causal and all-zero masks exactly.
  - softmax denominator: exp tiles accumulated on DVE into a float32r
    tile; column-sum and 1/d broadcast are two tiny full-rate f32r PE
    matmuls (ones vectors) reusing the outproj PSUM tag; no
    cross-partition reduce on the (collective-blocked) Pool engine.
  - v transposed to [sk, hd] tiles with dma_start_transpose (no PE/PSUM).
  - last block's output projection split into head-pair halves so half the
    contraction overlaps the last attention blocks; halves combined on DVE.
  - DMA issue spread across SP/ACT/gpsimd; x/w loads grouped 4 k-tiles per
    descriptor; final RS bounce split across three engines.

Matmuls in bf16 (fp32 PSUM accumulate). Output returned as bf16 row-shards
(rows 64c..64c+64 of each 512-row block), host converts to fp32.
"""

import math

import numpy as np

DIM = 4096
S = 2048
HD = 128
QH = 4           # q heads per core
NCORES = 8
KT = DIM // 128  # 32 contraction tiles
NSB = 4          # sq blocks
SBW = 512        # sq block width
SKT = S // 128   # 16 sk tiles
RSO = S // NSB // NCORES  # 64 rows per core per block after ReduceScatter

_CACHE = {}


def _mask_classes(m):
    """m: [sq, sk] fp32. Returns (cls, diag_lo): cls[sb][t] in
    {'skip','one','mult'}; for 'mult' tiles, diag_lo[sb][t] = largest
    128-multiple lo such that sq-cols [0, lo) are fully masked AND
    cols [lo+128, SBW) are fully unmasked (0 if no such narrowing)."""
    cls = []
    lo_tab = []
    for sb in range(NSB):
        row = []
        lrow = []
        sub_sq = m[sb * SBW:(sb + 1) * SBW]
        for t in range(SKT):
            sub = sub_sq[:, t * 128:(t + 1) * 128]  # [sq=512, sk=128]
            if np.all(sub < -1e8):
                row.append('skip')
                lrow.append(0)
            elif np.all(sub == 0.0):
                row.append('one')
                lrow.append(0)
            else:
                row.append('mult')
                lo = -1
                for cand in range(SBW - 128, -1, -128):
                    if (np.all(sub[:cand] < -1e8)
                            and np.all(sub[cand + 128:] == 0.0)):
                        lo = cand
                        break
                lrow.append(lo)
        # first live tile must start at col 0 (start=True zeroing + da init)
        first = next((t for t in range(SKT) if row[t] != 'skip'), None)
        if first is not None and row[first] == 'mult' and lrow[first] > 0:
            lrow[first] = -1
        cls.append(tuple(row))
        lo_tab.append(tuple(lrow))
    return tuple(cls), tuple(lo_tab)


def _build(mask_cls, diag_lo):
    import sys
    if '/opt/trn_rl_repo' not in sys.path:
        sys.path.insert(0, '/opt/trn_rl_repo')
    import concourse.bass as bass  # noqa: F401
    import concourse.mybir as mybir
    import concourse.tile as tile
    from concourse import bacc

    f32 = mybir.dt.float32
    bf16 = mybir.dt.bfloat16
    AF = mybir.ActivationFunctionType
    ALU = mybir.AluOpType

    nc = bacc.Bacc("TRN2", target_bir_lowering=False, debug=False,
                   enable_asserts=False, num_devices=NCORES)

    xT = nc.dram_tensor("xT", [DIM, S], bf16, kind="ExternalInput").ap()
    wq = nc.dram_tensor("wqkvT", [DIM, 6 * 128], bf16,
                        kind="ExternalInput").ap()
    woT = nc.dram_tensor("woT", [QH * 128, DIM], bf16,
                         kind="ExternalInput").ap()
    emaskT = nc.dram_tensor("emaskT", [S, S], bf16, kind="ExternalInput").ap()
    cosT = nc.dram_tensor("cosT", [HD, S], bf16, kind="ExternalInput").ap()
    sinT = nc.dram_tensor("sinT", [HD, S], bf16, kind="ExternalInput").ap()
    onesd_t = nc.dram_tensor("onesd", [128, 1], mybir.dt.float32r,
                             kind="ExternalInput").ap()
    onesr_t = nc.dram_tensor("onesr", [1, 128], mybir.dt.float32r,
                             kind="ExternalInput").ap()
    out = nc.dram_tensor("out", [NSB * RSO, DIM], bf16,
                         kind="ExternalOutput").ap()

    def _diag_lo(sb, t):
        return diag_lo[sb][t]

    # e-tile order within the qkv projection: k first, then v, then q0..q3
    # (k/v ready earliest so rope-k / v-transpose overlap the q matmuls).
    E_K, E_V = 0, 1
    E_Q0 = 2

    with tile.TileContext(nc) as tc:
        with (
            tc.tile_pool(name="pers", bufs=1) as pers,
            tc.tile_pool(name="wk", bufs=1) as wk,
            tc.tile_pool(name="ps", bufs=1, space="PSUM") as psp,
            tc.tile_pool(name="dram", bufs=1, space="DRAM") as dram,
        ):
            # ---- persistent SBUF state ----
            wts = pers.tile([128, KT * 6 * 128], bf16, name="wts")
            wq_r = wq.rearrange("(k p) e -> p k e", p=128)
            for g in range(8):
                nc.gpsimd.dma_start(
                    out=wts.rearrange("p (k e) -> p k e", k=KT)[:, 4 * g:4 * (g + 1), :],
                    in_=wq_r[:, 4 * g:4 * (g + 1), :])
            wo_sb = pers.tile([128, QH * DIM], bf16, name="wo_sb")
            for kk in range(QH):
                nc.gpsimd.dma_start(
                    out=wo_sb[:, kk * DIM:(kk + 1) * DIM],
                    in_=woT[kk * 128:(kk + 1) * 128, :])
            ctab = pers.tile([HD, S], bf16, name="ctab")
            nc.gpsimd.dma_start(out=ctab, in_=cosT)
            stab = pers.tile([HD, S], bf16, name="stab")
            nc.gpsimd.dma_start(out=stab, in_=sinT)

            kT = pers.tile([128, S], bf16, name="kT")
            vv = pers.tile([128, SKT * HD], bf16, name="vv")
            osbA_p = [pers.tile([128, DIM], bf16, name=f"osbAp{i}")
                      for i in range(2)]
            f32r = mybir.dt.float32r
            ones_d = pers.tile([128, 1], f32r, name="ones_d")
            nc.gpsimd.dma_start(out=ones_d, in_=onesd_t)
            ones_r = pers.tile([1, 128], f32r, name="ones_r")
            nc.gpsimd.dma_start(out=ones_r, in_=onesr_t)

            def rope(dst, ps, sb):
                """dst[:, :SBW] (SBUF bf16) = rope(ps) for sq block sb."""
                sl = slice(sb * SBW, (sb + 1) * SBW)
                qf = wk.tile([128, SBW], bf16, name="r_qf", tag="r_qf",
                             bufs=3)
                nc.scalar.activation(qf, ps, AF.Copy)
                qs = wk.tile([128, SBW], bf16, name="r_qs", tag="r_qs",
                             bufs=3)
                nc.sync.dma_start(out=qs[0:64, :], in_=qf[64:128, :])
                nc.sync.dma_start(out=qs[64:128, :], in_=qf[0:64, :])
                t1 = wk.tile([128, SBW], bf16, name="r_t1", tag="r_t1",
                             bufs=3)
                nc.vector.tensor_tensor(t1, qf, ctab[:, sl], ALU.mult)
                t2 = wk.tile([128, SBW], bf16, name="r_t2", tag="r_t2",
                             bufs=3)
                nc.vector.tensor_tensor(t2, qs, stab[:, sl], ALU.mult)
                nc.vector.tensor_tensor(dst, t1, t2, ALU.add)

            def qkv_gen(sb, q_sb):
                """project+rope block sb -> q_sb, kT/vv slices; yields at
                PE gaps."""
                x_sb = wk.tile([128, KT * SBW], bf16, name="x_sb", tag="x_sb",
                               bufs=1)
                xT_r = xT.rearrange("(k p) s -> p k s", p=128)
                for g in range(8):
                    nc.sync.dma_start(
                        out=x_sb.rearrange("p (k s) -> p k s", k=KT)[:, 4 * g:4 * (g + 1), :],
                        in_=xT_r[:, 4 * g:4 * (g + 1),
                                 sb * SBW:(sb + 1) * SBW])
                for e in range(6):
                    ps = psp.tile([128, SBW], f32, name="qps", tag="qps",
                                  bufs=2)
                    for k in range(KT):
                        nc.tensor.matmul(
                            ps, wts[:, k * 768 + e * 128:k * 768 + (e + 1) * 128],
                            x_sb[:, k * SBW:(k + 1) * SBW],
                            start=(k == 0), stop=(k == KT - 1))
                        if k % 2 == 1:
                            yield
                    if e == E_K:
                        rope(kT[:, sb * SBW:(sb + 1) * SBW], ps, sb)
                    elif e == E_V:
                        vt = wk.tile([128, SBW], bf16, name="vt", tag="vt",
                                     bufs=2)
                        nc.scalar.activation(vt, ps, AF.Copy)
                        for i in range(4):
                            skt = sb * 4 + i
                            nc.sync.dma_start_transpose(
                                vv[:, skt * HD:(skt + 1) * HD],
                                vt[:, i * 128:(i + 1) * 128])
                    else:
                        h = e - E_Q0
                        rope(q_sb[:, h * SBW:(h + 1) * SBW], ps, sb)

            def attn_gen(sb, q_sb, yn_sb, heads=range(QH)):
                """attention for block sb -> yn_sb; yields at PE gaps."""
                live = [t for t in range(SKT) if mask_cls[sb][t] != 'skip']
                # narrow partially-masked tiles to live cols when the mask
                # below the 128-col boundary is exactly all-pass (causal)
                los = {}
                mts = {}
                for t in live:
                    lo = 0
                    if mask_cls[sb][t] == 'mult':
                        dlo = _diag_lo(sb, t)
                        if dlo >= 0:
                            lo = dlo
                            mt = wk.tile([128, 128], bf16, name=f"mtn{t}",
                                         tag=f"mtn{t % 4}", bufs=2)
                            nc.sync.dma_start(
                                out=mt,
                                in_=emaskT[t * 128:(t + 1) * 128,
                                           sb * SBW + lo:sb * SBW + lo + 128])
                            mts[t] = (mt, slice(lo, lo + 128))
                        else:
                            mt = wk.tile([128, SBW], bf16, name=f"mtw{t}",
                                         tag=f"mtw{t % 4}", bufs=2)
                            nc.sync.dma_start(
                                out=mt,
                                in_=emaskT[t * 128:(t + 1) * 128,
                                           sb * SBW:(sb + 1) * SBW])
                            mts[t] = (mt, slice(0, SBW))
                    los[t] = lo
                for h in heads:
                    qsl = q_sb[:, h * SBW:(h + 1) * SBW]
                    yacc = psp.tile([128, SBW], f32, name="yacc", tag="yacc",
                                    bufs=2)
                    da = wk.tile([128, SBW], f32r, name="da", tag="da",
                                 bufs=2)
                    for j, t in enumerate(live):
                        # live column range of this tile: partially-masked
                        # causal diagonal tiles only need cols >= their sk
                        # offset (at 128 granularity); lo=0 for full tiles.
                        lo = los[t]
                        sl = slice(lo, SBW)
                        sps = psp.tile([128, SBW], f32, name="sps", tag="sps",
                                       bufs=2)
                        nc.tensor.matmul(sps[:, sl],
                                         kT[:, t * 128:(t + 1) * 128],
                                         qsl[:, sl], start=True, stop=True)
                        pt = wk.tile([128, SBW], bf16, name="pt", tag="pt",
                                     bufs=4)
                        nc.scalar.activation(pt[:, sl], sps[:, sl], AF.Exp)
                        if t in mts:
                            mt, msl = mts[t]
                            nc.vector.tensor_tensor(pt[:, msl], pt[:, msl],
                                                    mt, ALU.mult)
                        yield
                        nc.tensor.matmul(
                            yacc[:, sl], vv[:, t * HD:(t + 1) * HD],
                            pt[:, sl],
                            start=(j == 0), stop=(j == len(live) - 1),
                            skip_group_check=True)
                        with nc.allow_low_precision(
                                reason="f32r denominator accumulate"):
                            if j == 0:
                                nc.vector.tensor_copy(da, pt)
                            else:
                                nc.vector.tensor_tensor(
                                    da[:, sl], da[:, sl], pt[:, sl], ALU.add)
                        yield
                    # denominator: colsum + broadcast via tiny f32r matmuls
                    dtile = psp.tile([128, SBW], f32, name="dtile",
                                     tag="ops", bufs=2)
                    nc.tensor.matmul(dtile[0:1, :], ones_d,
                                     da, start=True, stop=True)
                    rec = wk.tile([1, SBW], f32r, name="rec", tag="rec",
                                  bufs=2)
                    with nc.allow_low_precision(
                            reason="f32r reciprocal of softmax denom"):
                        nc.vector.reciprocal(rec, dtile[0:1, :])
                    yield
                    btile = psp.tile([128, SBW], f32, name="btile",
                                     tag="ops", bufs=2)
                    nc.tensor.matmul(btile, ones_r,
                                     rec, start=True, stop=True)
                    bb = wk.tile([128, SBW], f32, name="bb", tag="bb",
                                 bufs=1)
                    nc.scalar.activation(bb, btile, AF.Copy)
                    nc.vector.tensor_tensor(
                        yn_sb[:, h * SBW:(h + 1) * SBW], yacc, bb,
                        ALU.mult)
                    yield

            def outproj_gen(sb, yn_sb, op_d):
                """partial out for block sb -> DRAM; yields at PE gaps."""
                for st in range(4):
                    osb = wk.tile([128, DIM], bf16, name="osb", tag="osb",
                                  bufs=2)
                    for oc in range(8):
                        ops = psp.tile([128, 512], f32, name="ops",
                                       tag="ops", bufs=2)
                        for kk in range(QH):
                            nc.tensor.matmul(
                                ops,
                                yn_sb[:, kk * SBW + st * 128:
                                      kk * SBW + (st + 1) * 128],
                                wo_sb[:, kk * DIM + oc * 512:
                                      kk * DIM + (oc + 1) * 512],
                                start=(kk == 0), stop=(kk == QH - 1))
                        if oc % 2 == 0:
                            nc.scalar.activation(
                                osb[:, oc * 512:(oc + 1) * 512], ops, AF.Copy)
                        else:
                            nc.vector.tensor_copy(
                                osb[:, oc * 512:(oc + 1) * 512], ops)
                        if oc == 3:
                            nc.sync.dma_start(
                                out=op_d[st * 128:(st + 1) * 128, 0:2048],
                                in_=osb[:, 0:2048])
                        yield
                    nc.sync.dma_start(
                        out=op_d[st * 128:(st + 1) * 128, 2048:4096],
                        in_=osb[:, 2048:4096])

            def outproj_half_gen(sb, yn_sb, kks, sts, stage, addin=None,
                                 op_d=None):
                """contraction over head subset kks for st tiles `sts`.
                stage: dict st->SBUF tile; A-pass (addin None) allocates
                into it, B-pass reads addin[st], combines, DMAs to op_d."""
                for st in sts:
                    if addin is None:
                        stage[st] = osbA_p[st % 2]
                    else:
                        stage[st] = wk.tile([128, DIM], bf16, name="osbF",
                                            tag="osb", bufs=2)
                    for oc in range(8):
                        ops = psp.tile([128, 512], f32, name="ops",
                                       tag="ops", bufs=2)
                        for i, kk in enumerate(kks):
                            nc.tensor.matmul(
                                ops,
                                yn_sb[:, kk * SBW + st * 128:
                                      kk * SBW + (st + 1) * 128],
                                wo_sb[:, kk * DIM + oc * 512:
                                      kk * DIM + (oc + 1) * 512],
                                start=(i == 0), stop=(i == len(kks) - 1))
                        osl = slice(oc * 512, (oc + 1) * 512)
                        if addin is not None:
                            nc.vector.tensor_tensor(
                                stage[st][:, osl], addin[st][:, osl], ops,
                                ALU.add)
                        elif oc % 2 == 0:
                            nc.scalar.activation(
                                stage[st][:, osl], ops, AF.Copy)
                        else:
                            nc.vector.tensor_copy(stage[st][:, osl], ops)
                        if addin is not None and oc == 3:
                            nc.sync.dma_start(
                                out=op_d[st * 128:(st + 1) * 128, 0:2048],
                                in_=stage[st][:, 0:2048])
                        yield
                    if addin is not None:
                        nc.sync.dma_start(
                            out=op_d[st * 128:(st + 1) * 128, 2048:4096],
                            in_=stage[st][:, 2048:4096])

            def rs(sb, op_d, last=False):
                rs_d = dram.tile([RSO, DIM], bf16, name="rs_d", tag="rs_d",
                                 bufs=2)
                nc.gpsimd.collective_compute(
                    "ReduceScatter",
                    ALU.add,
                    replica_groups=[list(range(NCORES))],
                    ins=[op_d.opt()],
                    outs=[rs_d.opt()],
                )
                if last:
                    cuts = [(0, 22, nc.sync), (22, 43, nc.scalar),
                            (43, RSO, nc.gpsimd)]
                    for a, b, eng in cuts:
                        eng.dma_start(
                            out=out[sb * RSO + a:sb * RSO + b, :],
                            in_=rs_d[a:b, :])
                else:
                    q = RSO // 4
                    for i in range(4):
                        nc.gpsimd.dma_start(
                            out=out[sb * RSO + i * q:sb * RSO + (i + 1) * q, :],
                            in_=rs_d[i * q:(i + 1) * q, :])

            def drive(*gens):
                """weighted round-robin over generators until drained."""
                gens = [g for g in gens if g is not None]
                while gens:
                    for g in list(gens):
                        try:
                            next(g)
                        except StopIteration:
                            gens.remove(g)

            # ---- the pipeline ----
            q_tiles = [wk.tile([128, QH * SBW], bf16, name="q_sb",
                               tag="q_sb", bufs=2) for _ in range(NSB)]
            yn_tiles = [wk.tile([128, QH * SBW], bf16, name="yn",
                                tag="yn", bufs=2) for _ in range(NSB)]
            opd_tiles = [dram.tile([SBW, DIM], bf16, name="op_d",
                                   tag="op_d", bufs=2) for _ in range(NSB)]
            L = NSB - 1
            osbA = {}
            osbF = {}
            drive(qkv_gen(0, q_tiles[0]))
            for sb in range(NSB - 1):
                drive(
                    attn_gen(sb, q_tiles[sb], yn_tiles[sb]),
                    qkv_gen(sb + 1, q_tiles[sb + 1]),
                    outproj_gen(sb - 1, yn_tiles[sb - 1], opd_tiles[sb - 1])
                    if sb > 0 else None,
                )
                if sb > 0:
                    rs(sb - 1, opd_tiles[sb - 1])
            # last block: heads 0/1 with outproj(L-1); heads 2/3 with the
            # kk={0,1} half-contraction; epilogue does kk={2,3} + combine.
            drive(
                attn_gen(L, q_tiles[L], yn_tiles[L], heads=range(2)),
                outproj_gen(L - 1, yn_tiles[L - 1], opd_tiles[L - 1]),
            )
            rs(L - 1, opd_tiles[L - 1])
            drive(
                attn_gen(L, q_tiles[L], yn_tiles[L], heads=range(2, QH)),
                outproj_half_gen(L, yn_tiles[L], [0, 1], [0, 1], osbA),
            )
            drive(
                outproj_half_gen(L, yn_tiles[L], [2, 3], [0, 1], osbF,
                                 addin=osbA, op_d=opd_tiles[L]),
                outproj_half_gen(L, yn_tiles[L], [0, 1], [2, 3], osbA),
            )
            drive(
                outproj_half_gen(L, yn_tiles[L], [2, 3], [2, 3], osbF,
                                 addin=osbA, op_d=opd_tiles[L]),
            )
            rs(L, opd_tiles[L], last=True)

    nc.finalize()
    return nc


def _prep_inputs(x, wqkv, wo, mask):
    import ml_dtypes
    bf = ml_dtypes.bfloat16

    x2 = np.ascontiguousarray(np.asarray(x, np.float32).reshape(S, DIM))
    xTh = np.ascontiguousarray(x2.T).astype(bf)

    m = np.asarray(mask, np.float32).reshape(S, S)
    emTh = np.exp(np.ascontiguousarray(m.T)).astype(bf)

    inv = 1.0 / (10000.0 ** (np.arange(0, HD, 2, dtype=np.float32)
                             / np.float32(HD)))
    tpos = np.arange(S, dtype=np.float32)
    freqs = np.outer(tpos, inv)
    emb = np.concatenate([freqs, freqs], axis=1)          # [S, 128]
    cosT = np.ascontiguousarray(np.cos(emb).T).astype(bf)  # [128, S]
    sinT = np.sin(emb).T
    sinmod = np.concatenate([-sinT[:64], sinT[64:]], axis=0)
    sinTh = np.ascontiguousarray(sinmod).astype(bf)
    scale = np.float32(1.0 / math.sqrt(HD))

    wqkv = np.asarray(wqkv, np.float32)
    wo = np.asarray(wo, np.float32)
    in_maps = []
    for c in range(NCORES):
        wq_c = np.concatenate([
            wqkv[4096 + 128 * c:4096 + 128 * (c + 1)],   # kv head c: k
            wqkv[5120 + 128 * c:5120 + 128 * (c + 1)],   # kv head c: v
            wqkv[512 * c:512 * (c + 1)] * scale,         # 4 q heads, scaled
        ], axis=0)                                        # [768, 4096]
        wq_cT = np.ascontiguousarray(wq_c.T).astype(bf)   # [4096, 768]
        # wo^T rows for this core's y features (wo columns 512c..512c+512)
        wo_cT = np.ascontiguousarray(
            wo[:, 512 * c:512 * (c + 1)].T).astype(bf)    # [512, 4096]
        in_maps.append({
            "xT": xTh, "wqkvT": wq_cT, "woT": wo_cT, "emaskT": emTh,
            "cosT": cosT, "sinT": sinTh,
            "onesd": np.ones((128, 1), np.float32),
            "onesr": np.ones((1, 128), np.float32),
        })
    return in_maps, m


OUT_NAMES = ["out"]


def _postprocess(res):
    full = np.empty((S, DIM), np.float32)
    for c in range(NCORES):
        oc = np.asarray(res[c]["out"]).astype(np.float32)
        oc = oc.reshape(NSB, RSO, DIM)
        for sb in range(NSB):
            full[sb * SBW + c * RSO: sb * SBW + (c + 1) * RSO] = oc[sb]
    return full.reshape(1, S, DIM)


def kernel(x, wqkv, wo, mask):
    import sys
    if '/opt/trn_rl_repo' not in sys.path:
        sys.path.insert(0, '/opt/trn_rl_repo')
    from concourse.bass_utils import run_bass_kernel_spmd

    in_maps, m = _prep_inputs(x, wqkv, wo, mask)
    cls, diag_lo = _mask_classes(m)
    key = (cls, diag_lo)
    if key not in _CACHE:
        _CACHE[key] = _build(cls, diag_lo)
    nc = _CACHE[key]

    res = run_bass_kernel_spmd(nc, in_maps, list(range(NCORES))).results
    return _postprocess(res)


# revision 16
# speedup vs baseline: 1.0144x; 1.0144x over previous
"""Llama GQA attention block (B=1, S=2048, D=4096, 32 Q heads / 8 KV heads,
hd=128) on 8 trn2 NeuronCores — v2.

Sharding: tensor-parallel by attention head. Core c owns q-heads 4c..4c+3 and
kv-head c (one GQA group). It computes the QKV projection for its 768 rows of
wqkv, RoPE, attention, then a K=512 PARTIAL output projection using only its
own attention output (y columns 512c..512c+512 against the matching 512 rows
of wo^T). Partials are summed across cores with a per-sq-block ReduceScatter
that overlaps the next block's compute; the host reassembles the row shards.

Key design points (vs the v1 baseline that AllGathered y and
column-sharded wo; sim cost model: 851us -> 413us):
  - No AllGather barrier, no 16MB y DRAM round-trip: y stays in SBUF and
    each per-block ReduceScatter overlaps the next block's compute on the
    collective cores. Only the last block's RS (~28us) is a serial tail.
  - Generator-driven instruction interleave: each phase round-robins the
    emission of attn(sb) with qkv(sb+1) and outproj(sb-1) so the in-order
    PE queue always has independent matmuls to fill exp/normalize latency
    gaps (also keeps the PE p-state ramped at full clock).
  - The Pool/gpsimd queue carries ONLY collectives + their bounce DMAs:
    a 28us ReduceScatter head-of-line blocks everything behind it on that
    engine, so no latency-critical work may queue there.
  - QKV PSUM rotates 2 banks (e-major loop over a pre-staged 4MB x block);
    PSUM budget = qkv 2 + scores 2 + yacc 2 + outproj/denominator 2 banks.
  - RoPE rotate-half via SBUF->SBUF DMA (sign folded into the sin table);
    rope math in bf16 on DVE (2x mode); 1/sqrt(hd) folded into wq; cos/sin
    tables shared between q and k.
  - mask applied multiplicatively AFTER exp (exp(s+m) = exp(s)*exp(m),
    exp(mask) precomputed on host in bf16): fully-masked [128,512] tiles
    skipped, all-zero tiles free, partially-masked causal-diagonal tiles
    narrowed to their live column range (compute, exp and dacc adds all
    shrink). Handles causal and all-zero masks exactly.
  - softmax denominator: exp tiles accumulated on DVE into a float32r
    tile; column-sum and 1/d broadcast are two tiny full-rate f32r PE
    matmuls (ones vectors) reusing the outproj PSUM tag; no
    cross-partition reduce on the (collective-blocked) Pool engine.
  - v transposed to [sk, hd] tiles with dma_start_transpose (no PE/PSUM).
  - last block's output projection split into head-pair halves so half the
    contraction overlaps the last attention blocks; halves combined on DVE.
  - DMA issue spread across SP/ACT/gpsimd; x/w loads grouped 4 k-tiles per
    descriptor; final RS bounce split across three engines.

Matmuls in bf16 (fp32 PSUM accumulate). Output returned as bf16 row-shards
(rows 64c..64c+64 of each 512-row block), host converts to fp32.
"""

import math

import numpy as np

DIM = 4096
S = 2048
HD = 128
QH = 4           # q heads per core
NCORES = 8
KT = DIM // 128  # 32 contraction tiles
NSB = 4          # sq blocks
SBW = 512        # sq block width
SKT = S // 128   # 16 sk tiles
RSO = S // NSB // NCORES  # 64 rows per core per block after ReduceScatter

_CACHE = {}


def _mask_classes(m):
    """m: [sq, sk] fp32. Returns (cls, diag_lo): cls[sb][t] in
    {'skip','one','mult'}; for 'mult' tiles, diag_lo[sb][t] = largest
    128-multiple lo such that sq-cols [0, lo) are fully masked AND
    cols [lo+128, SBW) are fully unmasked (0 if no such narrowing)."""
    cls = []
    lo_tab = []
    for sb in range(NSB):
        row = []
        lrow = []
        sub_sq = m[sb * SBW:(sb + 1) * SBW]
        for t in range(SKT):
            sub = sub_sq[:, t * 128:(t + 1) * 128]  # [sq=512, sk=128]
            if np.all(sub < -1e8):
                row.append('skip')
                lrow.append(0)
            elif np.all(sub == 0.0):
                row.append('one')
                lrow.append(0)
            else:
                row.append('mult')
                lo = -1
                for cand in range(SBW - 128, -1, -128):
                    if (np.all(sub[:cand] < -1e8)
                            and np.all(sub[cand + 128:] == 0.0)):
                        lo = cand
                        break
                lrow.append(lo)
        # first live tile must start at col 0 (start=True zeroing + da init)
        first = next((t for t in range(SKT) if row[t] != 'skip'), None)
        if first is not None and row[first] == 'mult' and lrow[first] > 0:
            lrow[first] = -1
        cls.append(tuple(row))
        lo_tab.append(tuple(lrow))
    return tuple(cls), tuple(lo_tab)


def _build(mask_cls, diag_lo):
    import sys
    if '/opt/trn_rl_repo' not in sys.path:
        sys.path.insert(0, '/opt/trn_rl_repo')
    import concourse.bass as bass  # noqa: F401
    import concourse.mybir as mybir
    import concourse.tile as tile
    from concourse import bacc

    f32 = mybir.dt.float32
    bf16 = mybir.dt.bfloat16
    AF = mybir.ActivationFunctionType
    ALU = mybir.AluOpType

    nc = bacc.Bacc("TRN2", target_bir_lowering=False, debug=False,
                   enable_asserts=False, num_devices=NCORES)

    xT = nc.dram_tensor("xT", [DIM, S], bf16, kind="ExternalInput").ap()
    wq = nc.dram_tensor("wqkvT", [DIM, 6 * 128], bf16,
                        kind="ExternalInput").ap()
    woT = nc.dram_tensor("woT", [QH * 128, DIM], bf16,
                         kind="ExternalInput").ap()
    emaskT = nc.dram_tensor("emaskT", [S, S], bf16, kind="ExternalInput").ap()
    cosT = nc.dram_tensor("cosT", [HD, S], bf16, kind="ExternalInput").ap()
    sinT = nc.dram_tensor("sinT", [HD, S], bf16, kind="ExternalInput").ap()
    onesd_t = nc.dram_tensor("onesd", [128, 1], mybir.dt.float32r,
                             kind="ExternalInput").ap()
    onesr_t = nc.dram_tensor("onesr", [1, 128], mybir.dt.float32r,
                             kind="ExternalInput").ap()
    out = nc.dram_tensor("out", [NSB * RSO, DIM], bf16,
                         kind="ExternalOutput").ap()

    def _diag_lo(sb, t):
        return diag_lo[sb][t]

    # e-tile order within the qkv projection: k first, then v, then q0..q3
    # (k/v ready earliest so rope-k / v-transpose overlap the q matmuls).
    E_K, E_V = 0, 1
    E_Q0 = 2

    with tile.TileContext(nc) as tc:
        with (
            tc.tile_pool(name="pers", bufs=1) as pers,
            tc.tile_pool(name="wk", bufs=1) as wk,
            tc.tile_pool(name="ps", bufs=1, space="PSUM") as psp,
            tc.tile_pool(name="dram", bufs=1, space="DRAM") as dram,
        ):
            # ---- persistent SBUF state ----
            wts = pers.tile([128, KT * 6 * 128], bf16, name="wts")
            wq_r = wq.rearrange("(k p) e -> p k e", p=128)
            kgroups = [(0, 1)] + [(1 + 4 * i, min(5 + 4 * i, KT))
                                  for i in range(8)]
            for a, b in kgroups:
                nc.gpsimd.dma_start(
                    out=wts.rearrange("p (k e) -> p k e", k=KT)[:, a:b, :],
                    in_=wq_r[:, a:b, :])
            wo_sb = pers.tile([128, QH * DIM], bf16, name="wo_sb")
            for kk in range(QH):
                nc.gpsimd.dma_start(
                    out=wo_sb[:, kk * DIM:(kk + 1) * DIM],
                    in_=woT[kk * 128:(kk + 1) * 128, :])
            ctab = pers.tile([HD, S], bf16, name="ctab")
            nc.gpsimd.dma_start(out=ctab, in_=cosT)
            stab = pers.tile([HD, S], bf16, name="stab")
            nc.gpsimd.dma_start(out=stab, in_=sinT)

            kT = pers.tile([128, S], bf16, name="kT")
            vv = pers.tile([128, SKT * HD], bf16, name="vv")
            osbA_p = [pers.tile([128, DIM], bf16, name=f"osbAp{i}")
                      for i in range(2)]
            f32r = mybir.dt.float32r
            ones_d = pers.tile([128, 1], f32r, name="ones_d")
            nc.gpsimd.dma_start(out=ones_d, in_=onesd_t)
            ones_r = pers.tile([1, 128], f32r, name="ones_r")
            nc.gpsimd.dma_start(out=ones_r, in_=onesr_t)

            def rope(dst, ps, sb):
                """dst[:, :SBW] (SBUF bf16) = rope(ps) for sq block sb."""
                sl = slice(sb * SBW, (sb + 1) * SBW)
                qf = wk.tile([128, SBW], bf16, name="r_qf", tag="r_qf",
                             bufs=3)
                nc.scalar.activation(qf, ps, AF.Copy)
                qs = wk.tile([128, SBW], bf16, name="r_qs", tag="r_qs",
                             bufs=3)
                nc.sync.dma_start(out=qs[0:64, :], in_=qf[64:128, :])
                nc.sync.dma_start(out=qs[64:128, :], in_=qf[0:64, :])
                t1 = wk.tile([128, SBW], bf16, name="r_t1", tag="r_t1",
                             bufs=3)
                nc.vector.tensor_tensor(t1, qf, ctab[:, sl], ALU.mult)
                t2 = wk.tile([128, SBW], bf16, name="r_t2", tag="r_t2",
                             bufs=3)
                nc.vector.tensor_tensor(t2, qs, stab[:, sl], ALU.mult)
                nc.vector.tensor_tensor(dst, t1, t2, ALU.add)

            def qkv_gen(sb, q_sb):
                """project+rope block sb -> q_sb, kT/vv slices; yields at
                PE gaps."""
                x_sb = wk.tile([128, KT * SBW], bf16, name="x_sb", tag="x_sb",
                               bufs=1)
                xT_r = xT.rearrange("(k p) s -> p k s", p=128)
                for a, b in kgroups:
                    nc.sync.dma_start(
                        out=x_sb.rearrange("p (k s) -> p k s", k=KT)[:, a:b, :],
                        in_=xT_r[:, a:b, sb * SBW:(sb + 1) * SBW])
                for e in range(6):
                    ps = psp.tile([128, SBW], f32, name="qps", tag="qps",
                                  bufs=2)
                    for k in range(KT):
                        nc.tensor.matmul(
                            ps, wts[:, k * 768 + e * 128:k * 768 + (e + 1) * 128],
                            x_sb[:, k * SBW:(k + 1) * SBW],
                            start=(k == 0), stop=(k == KT - 1))
                        if k % 2 == 1:
                            yield
                    if e == E_K:
                        rope(kT[:, sb * SBW:(sb + 1) * SBW], ps, sb)
                    elif e == E_V:
                        vt = wk.tile([128, SBW], bf16, name="vt", tag="vt",
                                     bufs=2)
                        nc.scalar.activation(vt, ps, AF.Copy)
                        for i in range(4):
                            skt = sb * 4 + i
                            nc.sync.dma_start_transpose(
                                vv[:, skt * HD:(skt + 1) * HD],
                                vt[:, i * 128:(i + 1) * 128])
                    else:
                        h = e - E_Q0
                        rope(q_sb[:, h * SBW:(h + 1) * SBW], ps, sb)

            def attn_gen(sb, q_sb, yn_sb, heads=range(QH)):
                """attention for block sb -> yn_sb; yields at PE gaps."""
                live = [t for t in range(SKT) if mask_cls[sb][t] != 'skip']
                # narrow partially-masked tiles to live cols when the mask
                # below the 128-col boundary is exactly all-pass (causal)
                los = {}
                mts = {}
                for t in live:
                    lo = 0
                    if mask_cls[sb][t] == 'mult':
                        dlo = _diag_lo(sb, t)
                        if dlo >= 0:
                            lo = dlo
                            mt = wk.tile([128, 128], bf16, name=f"mtn{t}",
                                         tag=f"mtn{t % 4}", bufs=2)
                            nc.sync.dma_start(
                                out=mt,
                                in_=emaskT[t * 128:(t + 1) * 128,
                                           sb * SBW + lo:sb * SBW + lo + 128])
                            mts[t] = (mt, slice(lo, lo + 128))
                        else:
                            mt = wk.tile([128, SBW], bf16, name=f"mtw{t}",
                                         tag=f"mtw{t % 4}", bufs=2)
                            nc.sync.dma_start(
                                out=mt,
                                in_=emaskT[t * 128:(t + 1) * 128,
                                           sb * SBW:(sb + 1) * SBW])
                            mts[t] = (mt, slice(0, SBW))
                    los[t] = lo
                for h in heads:
                    qsl = q_sb[:, h * SBW:(h + 1) * SBW]
                    yacc = psp.tile([128, SBW], f32, name="yacc", tag="yacc",
                                    bufs=2)
                    da = wk.tile([128, SBW], f32r, name="da", tag="da",
                                 bufs=2)
                    for j, t in enumerate(live):
                        # live column range of this tile: partially-masked
                        # causal diagonal tiles only need cols >= their sk
                        # offset (at 128 granularity); lo=0 for full tiles.
                        lo = los[t]
                        sl = slice(lo, SBW)
                        sps = psp.tile([128, SBW], f32, name="sps", tag="sps",
                                       bufs=2)
                        nc.tensor.matmul(sps[:, sl],
                                         kT[:, t * 128:(t + 1) * 128],
                                         qsl[:, sl], start=True, stop=True)
                        pt = wk.tile([128, SBW], bf16, name="pt", tag="pt",
                                     bufs=4)
                        nc.scalar.activation(pt[:, sl], sps[:, sl], AF.Exp)
                        if t in mts:
                            mt, msl = mts[t]
                            nc.vector.tensor_tensor(pt[:, msl], pt[:, msl],
                                                    mt, ALU.mult)
                        yield
                        nc.tensor.matmul(
                            yacc[:, sl], vv[:, t * HD:(t + 1) * HD],
                            pt[:, sl],
                            start=(j == 0), stop=(j == len(live) - 1),
                            skip_group_check=True)
                        with nc.allow_low_precision(
                                reason="f32r denominator accumulate"):
                            if j == 0:
                                nc.vector.tensor_copy(da, pt)
                            else:
                                nc.vector.tensor_tensor(
                                    da[:, sl], da[:, sl], pt[:, sl], ALU.add)
                        yield
                    # denominator: colsum + broadcast via tiny f32r matmuls
                    dtile = psp.tile([128, SBW], f32, name="dtile",
                                     tag="ops", bufs=2)
                    nc.tensor.matmul(dtile[0:1, :], ones_d,
                                     da, start=True, stop=True)
                    rec = wk.tile([1, SBW], f32r, name="rec", tag="rec",
                                  bufs=2)
                    with nc.allow_low_precision(
                            reason="f32r reciprocal of softmax denom"):
                        nc.vector.reciprocal(rec, dtile[0:1, :])
                    yield
                    btile = psp.tile([128, SBW], f32, name="btile",
                                     tag="ops", bufs=2)
                    nc.tensor.matmul(btile, ones_r,
                                     rec, start=True, stop=True)
                    bb = wk.tile([128, SBW], f32, name="bb", tag="bb",
                                 bufs=1)
                    nc.scalar.activation(bb, btile, AF.Copy)
                    nc.vector.tensor_tensor(
                        yn_sb[:, h * SBW:(h + 1) * SBW], yacc, bb,
                        ALU.mult)
                    yield

            def outproj_gen(sb, yn_sb, op_d, sts=range(4)):
                """partial out for block sb -> DRAM; yields at PE gaps."""
                for st in sts:
                    osb = wk.tile([128, DIM], bf16, name="osb", tag="osb",
                                  bufs=2)
                    for oc in range(8):
                        ops = psp.tile([128, 512], f32, name="ops",
                                       tag="ops", bufs=2)
                        for kk in range(QH):
                            nc.tensor.matmul(
                                ops,
                                yn_sb[:, kk * SBW + st * 128:
                                      kk * SBW + (st + 1) * 128],
                                wo_sb[:, kk * DIM + oc * 512:
                                      kk * DIM + (oc + 1) * 512],
                                start=(kk == 0), stop=(kk == QH - 1))
                        if oc % 2 == 0:
                            nc.scalar.activation(
                                osb[:, oc * 512:(oc + 1) * 512], ops, AF.Copy)
                        else:
                            nc.vector.tensor_copy(
                                osb[:, oc * 512:(oc + 1) * 512], ops)
                        if oc == 3:
                            nc.sync.dma_start(
                                out=op_d[st * 128:(st + 1) * 128, 0:2048],
                                in_=osb[:, 0:2048])
                        yield
                    nc.sync.dma_start(
                        out=op_d[st * 128:(st + 1) * 128, 2048:4096],
                        in_=osb[:, 2048:4096])

            def outproj_half_gen(sb, yn_sb, kks, sts, stage, addin=None,
                                 op_d=None):
                """contraction over head subset kks for st tiles `sts`.
                stage: dict st->SBUF tile; A-pass (addin None) allocates
                into it, B-pass reads addin[st], combines, DMAs to op_d."""
                for st in sts:
                    if addin is None:
                        stage[st] = osbA_p[st % 2]
                    else:
                        stage[st] = wk.tile([128, DIM], bf16, name="osbF",
                                            tag="osb", bufs=2)
                    for oc in range(8):
                        ops = psp.tile([128, 512], f32, name="ops",
                                       tag="ops", bufs=2)
                        for i, kk in enumerate(kks):
                            nc.tensor.matmul(
                                ops,
                                yn_sb[:, kk * SBW + st * 128:
                                      kk * SBW + (st + 1) * 128],
                                wo_sb[:, kk * DIM + oc * 512:
                                      kk * DIM + (oc + 1) * 512],
                                start=(i == 0), stop=(i == len(kks) - 1))
                        osl = slice(oc * 512, (oc + 1) * 512)
                        if addin is not None:
                            nc.vector.tensor_tensor(
                                stage[st][:, osl], addin[st][:, osl], ops,
                                ALU.add)
                        elif oc % 2 == 0:
                            nc.scalar.activation(
                                stage[st][:, osl], ops, AF.Copy)
                        else:
                            nc.vector.tensor_copy(stage[st][:, osl], ops)
                        if addin is not None and oc == 3:
                            nc.sync.dma_start(
                                out=op_d[st * 128:(st + 1) * 128, 0:2048],
                                in_=stage[st][:, 0:2048])
                        yield
                    if addin is not None:
                        nc.sync.dma_start(
                            out=op_d[st * 128:(st + 1) * 128, 2048:4096],
                            in_=stage[st][:, 2048:4096])

            def rs(sb, op_d, last=False):
                rs_d = dram.tile([RSO, DIM], bf16, name="rs_d", tag="rs_d",
                                 bufs=2)
                nc.gpsimd.collective_compute(
                    "ReduceScatter",
                    ALU.add,
                    replica_groups=[list(range(NCORES))],
                    ins=[op_d.opt()],
                    outs=[rs_d.opt()],
                )
                if last:
                    cuts = [(0, 22, nc.sync), (22, 43, nc.scalar),
                            (43, RSO, nc.gpsimd)]
                    for a, b, eng in cuts:
                        eng.dma_start(
                            out=out[sb * RSO + a:sb * RSO + b, :],
                            in_=rs_d[a:b, :])
                else:
                    q = RSO // 4
                    for i in range(4):
                        nc.gpsimd.dma_start(
                            out=out[sb * RSO + i * q:sb * RSO + (i + 1) * q, :],
                            in_=rs_d[i * q:(i + 1) * q, :])

            def drive(*specs):
                """weighted round-robin over (gen, weight) until drained.
                weight = steps advanced per round, sized so all generators
                finish together (minimizes single-stream drain at phase
                boundaries)."""
                pool = []
                for s in specs:
                    if s is None:
                        continue
                    g, w = s if isinstance(s, tuple) else (s, 1)
                    pool.append((g, w))
                while pool:
                    for g, w in list(pool):
                        for _ in range(w):
                            try:
                                next(g)
                            except StopIteration:
                                pool.remove((g, w))
                                break

            # ---- the pipeline ----
            q_tiles = [wk.tile([128, QH * SBW], bf16, name="q_sb",
                               tag="q_sb", bufs=2) for _ in range(NSB)]
            yn_tiles = [wk.tile([128, QH * SBW], bf16, name="yn",
                                tag="yn", bufs=2) for _ in range(NSB)]
            opd_tiles = [dram.tile([SBW, DIM], bf16, name="op_d",
                                   tag="op_d", bufs=2) for _ in range(NSB)]
            L = NSB - 1
            osbA = {}
            osbF = {}
            drive(qkv_gen(0, q_tiles[0]))
            # attn steps per block: 4 heads x (2L+2); qkv 96; outproj 32
            att_w = {0: 40, 1: 72, 2: 104}
            for sb in range(NSB - 1):
                aw = max(1, round(att_w[sb] / 33))
                drive(
                    (attn_gen(sb, q_tiles[sb], yn_tiles[sb]), aw),
                    (qkv_gen(sb + 1, q_tiles[sb + 1]), 3),
                    (outproj_gen(sb - 1, yn_tiles[sb - 1],
                                 opd_tiles[sb - 1]), 1)
                    if sb > 0 else None,
                )
                if sb > 0:
                    rs(sb - 1, opd_tiles[sb - 1])
            # last block: heads 0/1 with outproj(L-1); heads 2/3 with the
            # kk={0,1} half-contraction; then kk={2,3}+combine for st0/1
            # interleaved with plain full contraction for st2/3.
            drive(
                (attn_gen(L, q_tiles[L], yn_tiles[L], heads=range(2)), 2),
                (outproj_gen(L - 1, yn_tiles[L - 1], opd_tiles[L - 1]), 1),
            )
            rs(L - 1, opd_tiles[L - 1])
            drive(
                (attn_gen(L, q_tiles[L], yn_tiles[L], heads=range(2, QH)), 4),
                (outproj_half_gen(L, yn_tiles[L], [0, 1], [0, 1], osbA), 1),
            )
            drive(
                outproj_half_gen(L, yn_tiles[L], [2, 3], [0, 1], osbF,
                                 addin=osbA, op_d=opd_tiles[L]),
                outproj_gen(L, yn_tiles[L], opd_tiles[L], sts=[2, 3]),
            )
            rs(L, opd_tiles[L], last=True)

    nc.finalize()
    return nc


def _prep_inputs(x, wqkv, wo, mask):
    import ml_dtypes
    bf = ml_dtypes.bfloat16

    x2 = np.ascontiguousarray(np.asarray(x, np.float32).reshape(S, DIM))
    xTh = np.ascontiguousarray(x2.T).astype(bf)

    m = np.asarray(mask, np.float32).reshape(S, S)
    emTh = np.exp(np.ascontiguousarray(m.T)).astype(bf)

    inv = 1.0 / (10000.0 ** (np.arange(0, HD, 2, dtype=np.float32)
                             / np.float32(HD)))
    tpos = np.arange(S, dtype=np.float32)
    freqs = np.outer(tpos, inv)
    emb = np.concatenate([freqs, freqs], axis=1)          # [S, 128]
    cosT = np.ascontiguousarray(np.cos(emb).T).astype(bf)  # [128, S]
    sinT = np.sin(emb).T
    sinmod = np.concatenate([-sinT[:64], sinT[64:]], axis=0)
    sinTh = np.ascontiguousarray(sinmod).astype(bf)
    scale = np.float32(1.0 / math.sqrt(HD))

    wqkv = np.asarray(wqkv, np.float32)
    wo = np.asarray(wo, np.float32)
    in_maps = []
    for c in range(NCORES):
        wq_c = np.concatenate([
            wqkv[4096 + 128 * c:4096 + 128 * (c + 1)],   # kv head c: k
            wqkv[5120 + 128 * c:5120 + 128 * (c + 1)],   # kv head c: v
            wqkv[512 * c:512 * (c + 1)] * scale,         # 4 q heads, scaled
        ], axis=0)                                        # [768, 4096]
        wq_cT = np.ascontiguousarray(wq_c.T).astype(bf)   # [4096, 768]
        # wo^T rows for this core's y features (wo columns 512c..512c+512)
        wo_cT = np.ascontiguousarray(
            wo[:, 512 * c:512 * (c + 1)].T).astype(bf)    # [512, 4096]
        in_maps.append({
            "xT": xTh, "wqkvT": wq_cT, "woT": wo_cT, "emaskT": emTh,
            "cosT": cosT, "sinT": sinTh,
            "onesd": np.ones((128, 1), np.float32),
            "onesr": np.ones((1, 128), np.float32),
        })
    return in_maps, m


OUT_NAMES = ["out"]


def _postprocess(res):
    full = np.empty((S, DIM), np.float32)
    for c in range(NCORES):
        oc = np.asarray(res[c]["out"]).astype(np.float32)
        oc = oc.reshape(NSB, RSO, DIM)
        for sb in range(NSB):
            full[sb * SBW + c * RSO: sb * SBW + (c + 1) * RSO] = oc[sb]
    return full.reshape(1, S, DIM)


def kernel(x, wqkv, wo, mask):
    import sys
    if '/opt/trn_rl_repo' not in sys.path:
        sys.path.insert(0, '/opt/trn_rl_repo')
    from concourse.bass_utils import run_bass_kernel_spmd

    in_maps, m = _prep_inputs(x, wqkv, wo, mask)
    cls, diag_lo = _mask_classes(m)
    key = (cls, diag_lo)
    if key not in _CACHE:
        _CACHE[key] = _build(cls, diag_lo)
    nc = _CACHE[key]

    res = run_bass_kernel_spmd(nc, in_maps, list(range(NCORES))).results
    return _postprocess(res)


# revision 19
# speedup vs baseline: 1.0160x; 1.0015x over previous
"""Llama GQA attention block (B=1, S=2048, D=4096, 32 Q heads / 8 KV heads,
hd=128) on 8 trn2 NeuronCores — v2.

Sharding: tensor-parallel by attention head. Core c owns q-heads 4c..4c+3 and
kv-head c (one GQA group). It computes the QKV projection for its 768 rows of
wqkv, RoPE, attention, then a K=512 PARTIAL output projection using only its
own attention output (y columns 512c..512c+512 against the matching 512 rows
of wo^T). Partials are summed across cores with a per-sq-block ReduceScatter
that overlaps the next block's compute; the host reassembles the row shards.

Key design points (vs the v1 baseline that AllGathered y and
column-sharded wo; sim cost model: 851us -> 407us):
  - No AllGather barrier, no 16MB y DRAM round-trip: y stays in SBUF and
    each per-block ReduceScatter overlaps the next block's compute on the
    collective cores. Only the last block's RS (~28us) is a serial tail.
  - Generator-driven instruction interleave: each phase round-robins the
    emission of attn(sb) with qkv(sb+1) and outproj(sb-1) so the in-order
    PE queue always has independent matmuls to fill exp/normalize latency
    gaps (also keeps the PE p-state ramped at full clock).
  - The Pool/gpsimd queue carries ONLY collectives + their bounce DMAs:
    a 28us ReduceScatter head-of-line blocks everything behind it on that
    engine, so no latency-critical work may queue there.
  - QKV PSUM rotates 2 banks (e-major loop over a pre-staged 4MB x block);
    PSUM budget = qkv 2 + scores 2 + yacc 2 + outproj/denominator 2 banks.
  - RoPE rotate-half via SBUF->SBUF DMA (sign folded into the sin table);
    rope math in bf16 on DVE (2x mode); 1/sqrt(hd) folded into wq; cos/sin
    tables shared between q and k.
  - mask applied multiplicatively AFTER exp (exp(s+m) = exp(s)*exp(m),
    exp(mask) precomputed on host in bf16): fully-masked [128,512] tiles
    skipped, all-zero tiles free, partially-masked causal-diagonal tiles
    narrowed to their live column range (compute, exp and dacc adds all
    shrink). Handles causal and all-zero masks exactly.
  - softmax denominator: exp tiles accumulated on DVE into a float32r
    tile; column-sum and 1/d broadcast are two tiny full-rate f32r PE
    matmuls (ones vectors) reusing the outproj PSUM tag; no
    cross-partition reduce on the (collective-blocked) Pool engine.
  - v transposed to [sk, hd] tiles with dma_start_transpose (no PE/PSUM).
  - last block's output projection: st0/1 contract heads 0/1 during the
    last attention heads, then combine with heads 2/3 on DVE while st2/3
    run as plain full contractions -- no separate epilogue drain.
  - DMA issue spread across SP/ACT/gpsimd; x/w loads grouped 4 k-tiles per
    descriptor; final RS bounce split across three engines.

Matmuls in bf16 (fp32 PSUM accumulate). Output returned as bf16 row-shards
(rows 64c..64c+64 of each 512-row block), host converts to fp32.
"""

import math

import numpy as np

DIM = 4096
S = 2048
HD = 128
QH = 4           # q heads per core
NCORES = 8
KT = DIM // 128  # 32 contraction tiles
NSB = 4          # sq blocks
SBW = 512        # sq block width
SKT = S // 128   # 16 sk tiles
RSO = S // NSB // NCORES  # 64 rows per core per block after ReduceScatter

_CACHE = {}


def _mask_classes(m):
    """m: [sq, sk] fp32. Returns (cls, diag_lo): cls[sb][t] in
    {'skip','one','mult'}; for 'mult' tiles, diag_lo[sb][t] = largest
    128-multiple lo such that sq-cols [0, lo) are fully masked AND
    cols [lo+128, SBW) are fully unmasked (0 if no such narrowing)."""
    cls = []
    lo_tab = []
    for sb in range(NSB):
        row = []
        lrow = []
        sub_sq = m[sb * SBW:(sb + 1) * SBW]
        for t in range(SKT):
            sub = sub_sq[:, t * 128:(t + 1) * 128]  # [sq=512, sk=128]
            if np.all(sub < -1e8):
                row.append('skip')
                lrow.append(0)
            elif np.all(sub == 0.0):
                row.append('one')
                lrow.append(0)
            else:
                row.append('mult')
                lo = -1
                for cand in range(SBW - 128, -1, -128):
                    if (np.all(sub[:cand] < -1e8)
                            and np.all(sub[cand + 128:] == 0.0)):
                        lo = cand
                        break
                lrow.append(lo)
        # first live tile must start at col 0 (start=True zeroing + da init)
        first = next((t for t in range(SKT) if row[t] != 'skip'), None)
        if first is not None and row[first] == 'mult' and lrow[first] > 0:
            lrow[first] = -1
        cls.append(tuple(row))
        lo_tab.append(tuple(lrow))
    return tuple(cls), tuple(lo_tab)


def _build(mask_cls, diag_lo):
    import sys
    if '/opt/trn_rl_repo' not in sys.path:
        sys.path.insert(0, '/opt/trn_rl_repo')
    import concourse.bass as bass  # noqa: F401
    import concourse.mybir as mybir
    import concourse.tile as tile
    from concourse import bacc

    f32 = mybir.dt.float32
    bf16 = mybir.dt.bfloat16
    AF = mybir.ActivationFunctionType
    ALU = mybir.AluOpType

    nc = bacc.Bacc("TRN2", target_bir_lowering=False, debug=False,
                   enable_asserts=False, num_devices=NCORES)

    xT = nc.dram_tensor("xT", [DIM, S], bf16, kind="ExternalInput").ap()
    wq = nc.dram_tensor("wqkvT", [DIM, 6 * 128], bf16,
                        kind="ExternalInput").ap()
    woT = nc.dram_tensor("woT", [QH * 128, DIM], bf16,
                         kind="ExternalInput").ap()
    emaskT = nc.dram_tensor("emaskT", [S, S], bf16, kind="ExternalInput").ap()
    cosT = nc.dram_tensor("cosT", [HD, S], bf16, kind="ExternalInput").ap()
    sinT = nc.dram_tensor("sinT", [HD, S], bf16, kind="ExternalInput").ap()
    onesd_t = nc.dram_tensor("onesd", [128, 1], mybir.dt.float32r,
                             kind="ExternalInput").ap()
    onesr_t = nc.dram_tensor("onesr", [1, 128], mybir.dt.float32r,
                             kind="ExternalInput").ap()
    out = nc.dram_tensor("out", [NSB * RSO, DIM], bf16,
                         kind="ExternalOutput").ap()

    def _diag_lo(sb, t):
        return diag_lo[sb][t]

    # e-tile order within the qkv projection: k first, then v, then q0..q3
    # (k/v ready earliest so rope-k / v-transpose overlap the q matmuls).
    E_K, E_V = 0, 1
    E_Q0 = 2

    with tile.TileContext(nc) as tc:
        with (
            tc.tile_pool(name="pers", bufs=1) as pers,
            tc.tile_pool(name="wk", bufs=1) as wk,
            tc.tile_pool(name="ps", bufs=1, space="PSUM") as psp,
            tc.tile_pool(name="dram", bufs=1, space="DRAM") as dram,
        ):
            # ---- persistent SBUF state ----
            wts = pers.tile([128, KT * 6 * 128], bf16, name="wts")
            wq_r = wq.rearrange("(k p) e -> p k e", p=128)
            kgroups = [(0, 1)] + [(1 + 4 * i, min(5 + 4 * i, KT))
                                  for i in range(8)]
            for gi, (a, b) in enumerate(kgroups):
                eng = nc.gpsimd if gi % 2 == 0 else nc.scalar
                eng.dma_start(
                    out=wts.rearrange("p (k e) -> p k e", k=KT)[:, a:b, :],
                    in_=wq_r[:, a:b, :])
            wo_sb = pers.tile([128, QH * DIM], bf16, name="wo_sb")
            for kk in range(QH):
                nc.gpsimd.dma_start(
                    out=wo_sb[:, kk * DIM:(kk + 1) * DIM],
                    in_=woT[kk * 128:(kk + 1) * 128, :])
            ctab = pers.tile([HD, S], bf16, name="ctab")
            nc.gpsimd.dma_start(out=ctab, in_=cosT)
            stab = pers.tile([HD, S], bf16, name="stab")
            nc.gpsimd.dma_start(out=stab, in_=sinT)

            kT = pers.tile([128, S], bf16, name="kT")
            vv = pers.tile([128, SKT * HD], bf16, name="vv")
            osbA_p = [pers.tile([128, DIM], bf16, name=f"osbAp{i}")
                      for i in range(2)]
            f32r = mybir.dt.float32r
            ones_d = pers.tile([128, 1], f32r, name="ones_d")
            nc.gpsimd.dma_start(out=ones_d, in_=onesd_t)
            ones_r = pers.tile([1, 128], f32r, name="ones_r")
            nc.gpsimd.dma_start(out=ones_r, in_=onesr_t)

            def rope(dst, ps, sb):
                """dst[:, :SBW] (SBUF bf16) = rope(ps) for sq block sb."""
                sl = slice(sb * SBW, (sb + 1) * SBW)
                qf = wk.tile([128, SBW], bf16, name="r_qf", tag="r_qf",
                             bufs=3)
                nc.scalar.activation(qf, ps, AF.Copy)
                qs = wk.tile([128, SBW], bf16, name="r_qs", tag="r_qs",
                             bufs=3)
                nc.sync.dma_start(out=qs[0:64, :], in_=qf[64:128, :])
                nc.sync.dma_start(out=qs[64:128, :], in_=qf[0:64, :])
                t1 = wk.tile([128, SBW], bf16, name="r_t1", tag="r_t1",
                             bufs=3)
                nc.vector.tensor_tensor(t1, qf, ctab[:, sl], ALU.mult)
                t2 = wk.tile([128, SBW], bf16, name="r_t2", tag="r_t2",
                             bufs=3)
                nc.vector.tensor_tensor(t2, qs, stab[:, sl], ALU.mult)
                nc.vector.tensor_tensor(dst, t1, t2, ALU.add)

            def qkv_gen(sb, q_sb):
                """project+rope block sb -> q_sb, kT/vv slices; yields at
                PE gaps."""
                x_sb = wk.tile([128, KT * SBW], bf16, name="x_sb", tag="x_sb",
                               bufs=1)
                xT_r = xT.rearrange("(k p) s -> p k s", p=128)
                for a, b in kgroups:
                    nc.sync.dma_start(
                        out=x_sb.rearrange("p (k s) -> p k s", k=KT)[:, a:b, :],
                        in_=xT_r[:, a:b, sb * SBW:(sb + 1) * SBW])
                for e in range(6):
                    ps = psp.tile([128, SBW], f32, name="qps", tag="qps",
                                  bufs=2)
                    for k in range(KT):
                        nc.tensor.matmul(
                            ps, wts[:, k * 768 + e * 128:k * 768 + (e + 1) * 128],
                            x_sb[:, k * SBW:(k + 1) * SBW],
                            start=(k == 0), stop=(k == KT - 1))
                        if k % 2 == 1:
                            yield
                    if e == E_K:
                        rope(kT[:, sb * SBW:(sb + 1) * SBW], ps, sb)
                    elif e == E_V:
                        vt = wk.tile([128, SBW], bf16, name="vt", tag="vt",
                                     bufs=2)
                        nc.scalar.activation(vt, ps, AF.Copy)
                        for i in range(4):
                            skt = sb * 4 + i
                            nc.sync.dma_start_transpose(
                                vv[:, skt * HD:(skt + 1) * HD],
                                vt[:, i * 128:(i + 1) * 128])
                    else:
                        h = e - E_Q0
                        rope(q_sb[:, h * SBW:(h + 1) * SBW], ps, sb)

            def attn_gen(sb, q_sb, yn_sb, heads=range(QH)):
                """attention for block sb -> yn_sb; yields at PE gaps."""
                live = [t for t in range(SKT) if mask_cls[sb][t] != 'skip']
                # narrow partially-masked tiles to live cols when the mask
                # below the 128-col boundary is exactly all-pass (causal)
                los = {}
                mts = {}
                for t in live:
                    lo = 0
                    if mask_cls[sb][t] == 'mult':
                        dlo = _diag_lo(sb, t)
                        if dlo >= 0:
                            lo = dlo
                            mt = wk.tile([128, 128], bf16, name=f"mtn{t}",
                                         tag=f"mtn{t % 4}", bufs=2)
                            nc.sync.dma_start(
                                out=mt,
                                in_=emaskT[t * 128:(t + 1) * 128,
                                           sb * SBW + lo:sb * SBW + lo + 128])
                            mts[t] = (mt, slice(lo, lo + 128))
                        else:
                            mt = wk.tile([128, SBW], bf16, name=f"mtw{t}",
                                         tag=f"mtw{t % 4}", bufs=2)
                            nc.sync.dma_start(
                                out=mt,
                                in_=emaskT[t * 128:(t + 1) * 128,
                                           sb * SBW:(sb + 1) * SBW])
                            mts[t] = (mt, slice(0, SBW))
                    los[t] = lo
                for h in heads:
                    qsl = q_sb[:, h * SBW:(h + 1) * SBW]
                    yacc = psp.tile([128, SBW], f32, name="yacc", tag="yacc",
                                    bufs=2)
                    da = wk.tile([128, SBW], f32r, name="da", tag="da",
                                 bufs=2)
                    for j, t in enumerate(live):
                        # live column range of this tile: partially-masked
                        # causal diagonal tiles only need cols >= their sk
                        # offset (at 128 granularity); lo=0 for full tiles.
                        lo = los[t]
                        sl = slice(lo, SBW)
                        sps = psp.tile([128, SBW], f32, name="sps", tag="sps",
                                       bufs=2)
                        nc.tensor.matmul(sps[:, sl],
                                         kT[:, t * 128:(t + 1) * 128],
                                         qsl[:, sl], start=True, stop=True)
                        pt = wk.tile([128, SBW], bf16, name="pt", tag="pt",
                                     bufs=4)
                        nc.scalar.activation(pt[:, sl], sps[:, sl], AF.Exp)
                        if t in mts:
                            mt, msl = mts[t]
                            nc.vector.tensor_tensor(pt[:, msl], pt[:, msl],
                                                    mt, ALU.mult)
                        yield
                        nc.tensor.matmul(
                            yacc[:, sl], vv[:, t * HD:(t + 1) * HD],
                            pt[:, sl],
                            start=(j == 0), stop=(j == len(live) - 1),
                            skip_group_check=True)
                        with nc.allow_low_precision(
                                reason="f32r denominator accumulate"):
                            if j == 0:
                                nc.vector.tensor_copy(da, pt)
                            else:
                                nc.vector.tensor_tensor(
                                    da[:, sl], da[:, sl], pt[:, sl], ALU.add)
                        yield
                    # denominator: colsum + broadcast via tiny f32r matmuls
                    dtile = psp.tile([128, SBW], f32, name="dtile",
                                     tag="ops", bufs=2)
                    nc.tensor.matmul(dtile[0:1, :], ones_d,
                                     da, start=True, stop=True)
                    rec = wk.tile([1, SBW], f32r, name="rec", tag="rec",
                                  bufs=2)
                    with nc.allow_low_precision(
                            reason="f32r reciprocal of softmax denom"):
                        nc.vector.reciprocal(rec, dtile[0:1, :])
                    yield
                    btile = psp.tile([128, SBW], f32, name="btile",
                                     tag="ops", bufs=2)
                    nc.tensor.matmul(btile, ones_r,
                                     rec, start=True, stop=True)
                    bb = wk.tile([128, SBW], f32, name="bb", tag="bb",
                                 bufs=1)
                    nc.scalar.activation(bb, btile, AF.Copy)
                    nc.vector.tensor_tensor(
                        yn_sb[:, h * SBW:(h + 1) * SBW], yacc, bb,
                        ALU.mult)
                    yield

            def outproj_gen(sb, yn_sb, op_d, sts=range(4)):
                """partial out for block sb -> DRAM; yields at PE gaps."""
                for st in sts:
                    osb = wk.tile([128, DIM], bf16, name="osb", tag="osb",
                                  bufs=2)
                    for oc in range(8):
                        ops = psp.tile([128, 512], f32, name="ops",
                                       tag="ops", bufs=2)
                        for kk in range(QH):
                            nc.tensor.matmul(
                                ops,
                                yn_sb[:, kk * SBW + st * 128:
                                      kk * SBW + (st + 1) * 128],
                                wo_sb[:, kk * DIM + oc * 512:
                                      kk * DIM + (oc + 1) * 512],
                                start=(kk == 0), stop=(kk == QH - 1))
                        if oc % 2 == 0:
                            nc.scalar.activation(
                                osb[:, oc * 512:(oc + 1) * 512], ops, AF.Copy)
                        else:
                            nc.vector.tensor_copy(
                                osb[:, oc * 512:(oc + 1) * 512], ops)
                        if oc == 3:
                            nc.sync.dma_start(
                                out=op_d[st * 128:(st + 1) * 128, 0:2048],
                                in_=osb[:, 0:2048])
                        yield
                    nc.sync.dma_start(
                        out=op_d[st * 128:(st + 1) * 128, 2048:4096],
                        in_=osb[:, 2048:4096])

            def outproj_half_gen(sb, yn_sb, kks, sts, stage, addin=None,
                                 op_d=None):
                """contraction over head subset kks for st tiles `sts`.
                stage: dict st->SBUF tile; A-pass (addin None) allocates
                into it, B-pass reads addin[st], combines, DMAs to op_d."""
                for st in sts:
                    if addin is None:
                        stage[st] = osbA_p[st % 2]
                    else:
                        stage[st] = wk.tile([128, DIM], bf16, name="osbF",
                                            tag="osb", bufs=2)
                    for oc in range(8):
                        ops = psp.tile([128, 512], f32, name="ops",
                                       tag="ops", bufs=2)
                        for i, kk in enumerate(kks):
                            nc.tensor.matmul(
                                ops,
                                yn_sb[:, kk * SBW + st * 128:
                                      kk * SBW + (st + 1) * 128],
                                wo_sb[:, kk * DIM + oc * 512:
                                      kk * DIM + (oc + 1) * 512],
                                start=(i == 0), stop=(i == len(kks) - 1))
                        osl = slice(oc * 512, (oc + 1) * 512)
                        if addin is not None:
                            nc.vector.tensor_tensor(
                                stage[st][:, osl], addin[st][:, osl], ops,
                                ALU.add)
                        elif oc % 2 == 0:
                            nc.scalar.activation(
                                stage[st][:, osl], ops, AF.Copy)
                        else:
                            nc.vector.tensor_copy(stage[st][:, osl], ops)
                        if addin is not None and oc == 3:
                            nc.sync.dma_start(
                                out=op_d[st * 128:(st + 1) * 128, 0:2048],
                                in_=stage[st][:, 0:2048])
                        yield
                    if addin is not None:
                        nc.sync.dma_start(
                            out=op_d[st * 128:(st + 1) * 128, 2048:4096],
                            in_=stage[st][:, 2048:4096])

            def rs(sb, op_d, last=False):
                rs_d = dram.tile([RSO, DIM], bf16, name="rs_d", tag="rs_d",
                                 bufs=2)
                nc.gpsimd.collective_compute(
                    "ReduceScatter",
                    ALU.add,
                    replica_groups=[list(range(NCORES))],
                    ins=[op_d.opt()],
                    outs=[rs_d.opt()],
                )
                if last:
                    cuts = [(0, 22, nc.sync), (22, 43, nc.scalar),
                            (43, RSO, nc.gpsimd)]
                    for a, b, eng in cuts:
                        eng.dma_start(
                            out=out[sb * RSO + a:sb * RSO + b, :],
                            in_=rs_d[a:b, :])
                else:
                    q = RSO // 4
                    for i in range(4):
                        nc.gpsimd.dma_start(
                            out=out[sb * RSO + i * q:sb * RSO + (i + 1) * q, :],
                            in_=rs_d[i * q:(i + 1) * q, :])

            def drive(*specs):
                """weighted round-robin over (gen, weight) until drained.
                weight = steps advanced per round, sized so all generators
                finish together (minimizes single-stream drain at phase
                boundaries)."""
                pool = []
                for s in specs:
                    if s is None:
                        continue
                    g, w = s if isinstance(s, tuple) else (s, 1)
                    pool.append((g, w))
                while pool:
                    for g, w in list(pool):
                        for _ in range(w):
                            try:
                                next(g)
                            except StopIteration:
                                pool.remove((g, w))
                                break

            # ---- the pipeline ----
            q_tiles = [wk.tile([128, QH * SBW], bf16, name="q_sb",
                               tag="q_sb", bufs=2) for _ in range(NSB)]
            yn_tiles = [wk.tile([128, QH * SBW], bf16, name="yn",
                                tag="yn", bufs=2) for _ in range(NSB)]
            opd_tiles = [dram.tile([SBW, DIM], bf16, name="op_d",
                                   tag="op_d", bufs=2) for _ in range(NSB)]
            L = NSB - 1
            osbA = {}
            osbF = {}
            drive(qkv_gen(0, q_tiles[0]))
            # attn steps per block: 4 heads x (2L+2); qkv 96; outproj 32
            att_w = {0: 40, 1: 72, 2: 104}
            for sb in range(NSB - 1):
                aw = max(1, round(att_w[sb] / 33))
                drive(
                    (attn_gen(sb, q_tiles[sb], yn_tiles[sb]), aw),
                    (qkv_gen(sb + 1, q_tiles[sb + 1]), 3),
                    (outproj_gen(sb - 1, yn_tiles[sb - 1],
                                 opd_tiles[sb - 1]), 1)
                    if sb > 0 else None,
                )
                if sb > 0:
                    rs(sb - 1, opd_tiles[sb - 1])
            # last block: heads 0/1 with outproj(L-1); heads 2/3 with the
            # kk={0,1} half-contraction; then kk={2,3}+combine for st0/1
            # interleaved with plain full contraction for st2/3.
            drive(
                (attn_gen(L, q_tiles[L], yn_tiles[L], heads=range(2)), 2),
                (outproj_gen(L - 1, yn_tiles[L - 1], opd_tiles[L - 1]), 1),
            )
            rs(L - 1, opd_tiles[L - 1])
            drive(
                (attn_gen(L, q_tiles[L], yn_tiles[L], heads=range(2, QH)), 4),
                (outproj_half_gen(L, yn_tiles[L], [0, 1], [0, 1], osbA), 1),
            )
            drive(
                outproj_half_gen(L, yn_tiles[L], [2, 3], [0, 1], osbF,
                                 addin=osbA, op_d=opd_tiles[L]),
                outproj_gen(L, yn_tiles[L], opd_tiles[L], sts=[2, 3]),
            )
            rs(L, opd_tiles[L], last=True)

    nc.finalize()
    return nc


def _prep_inputs(x, wqkv, wo, mask):
    import ml_dtypes
    bf = ml_dtypes.bfloat16

    x2 = np.ascontiguousarray(np.asarray(x, np.float32).reshape(S, DIM))
    xTh = np.ascontiguousarray(x2.T).astype(bf)

    m = np.asarray(mask, np.float32).reshape(S, S)
    emTh = np.exp(np.ascontiguousarray(m.T)).astype(bf)

    inv = 1.0 / (10000.0 ** (np.arange(0, HD, 2, dtype=np.float32)
                             / np.float32(HD)))
    tpos = np.arange(S, dtype=np.float32)
    freqs = np.outer(tpos, inv)
    emb = np.concatenate([freqs, freqs], axis=1)          # [S, 128]
    cosT = np.ascontiguousarray(np.cos(emb).T).astype(bf)  # [128, S]
    sinT = np.sin(emb).T
    sinmod = np.concatenate([-sinT[:64], sinT[64:]], axis=0)
    sinTh = np.ascontiguousarray(sinmod).astype(bf)
    scale = np.float32(1.0 / math.sqrt(HD))

    wqkv = np.asarray(wqkv, np.float32)
    wo = np.asarray(wo, np.float32)
    in_maps = []
    for c in range(NCORES):
        wq_c = np.concatenate([
            wqkv[4096 + 128 * c:4096 + 128 * (c + 1)],   # kv head c: k
            wqkv[5120 + 128 * c:5120 + 128 * (c + 1)],   # kv head c: v
            wqkv[512 * c:512 * (c + 1)] * scale,         # 4 q heads, scaled
        ], axis=0)                                        # [768, 4096]
        wq_cT = np.ascontiguousarray(wq_c.T).astype(bf)   # [4096, 768]
        # wo^T rows for this core's y features (wo columns 512c..512c+512)
        wo_cT = np.ascontiguousarray(
            wo[:, 512 * c:512 * (c + 1)].T).astype(bf)    # [512, 4096]
        in_maps.append({
            "xT": xTh, "wqkvT": wq_cT, "woT": wo_cT, "emaskT": emTh,
            "cosT": cosT, "sinT": sinTh,
            "onesd": np.ones((128, 1), np.float32),
            "onesr": np.ones((1, 128), np.float32),
        })
    return in_maps, m


OUT_NAMES = ["out"]


def _postprocess(res):
    full = np.empty((S, DIM), np.float32)
    for c in range(NCORES):
        oc = np.asarray(res[c]["out"]).astype(np.float32)
        oc = oc.reshape(NSB, RSO, DIM)
        for sb in range(NSB):
            full[sb * SBW + c * RSO: sb * SBW + (c + 1) * RSO] = oc[sb]
    return full.reshape(1, S, DIM)


def kernel(x, wqkv, wo, mask):
    import sys
    if '/opt/trn_rl_repo' not in sys.path:
        sys.path.insert(0, '/opt/trn_rl_repo')
    from concourse.bass_utils import run_bass_kernel_spmd

    in_maps, m = _prep_inputs(x, wqkv, wo, mask)
    cls, diag_lo = _mask_classes(m)
    key = (cls, diag_lo)
    if key not in _CACHE:
        _CACHE[key] = _build(cls, diag_lo)
    nc = _CACHE[key]

    res = run_bass_kernel_spmd(nc, in_maps, list(range(NCORES))).results
    return _postprocess(res)
